# revision 2
# baseline (speedup 1.0000x reference)
"""Trainium2 Bass kernel for nn_ContextEncoderEMA — v8.

v6/v7 history: fp8 DoubleRow indicator-matmul, host-folded weights
(one fp8 quant of w_r*e_r), EMA-only rows resharded by row count,
host applies seed/last/tail exactly.  v7 = 66 us: scalar-engine
copies starved its DMA doorbells; 28 fixed supertile strips shipped
2.75 MB of output.

v8 changes:
  * Variable strips: greedy-common grouping of 256-row m-blocks
    (3,3,...,2 per strip, 19 strips/core, all cores share the program
    structure; per-core fragment offsets live in S).  Output drops to
    1.87 MB/core, copies drop 28 -> 19.
  * Copies alternate vector/gpsimd; scalar+sync only issue HW-DGE DMA
    doorbells (no compute), stores on gpsimd.
  * S in 14 per-load chunks issued just before each load on the same
    queue: first matmul gates only on chunk0+load0.

Per-core HBM: 11.0 in + 0.92 S + 1.87 out = 13.8 MB @ ~16x26 GB/s.
"""

import numpy as np

TAU = np.float32(0.9)
D = 768
N_CORES = 8
P = 128
SLOT = 64               # fragment slots per strip (psum partitions)
R_CORE = 14336          # EMA rows per core
NBLK = R_CORE // 256    # 56 m-blocks of 256 rows
NLOADS = R_CORE // 1024  # 14 loads of 1024 rows (4 m-blocks)
S_COLS = NBLK * 2 * SLOT  # 7168

_cache = {}


def _build_program(strip_blocks):
    import concourse.bacc as bacc
    import concourse.mybir as mybir
    from concourse.tile import TileContext

    f32 = mybir.dt.float32
    bf16 = mybir.dt.bfloat16
    f8 = mybir.dt.float8e4
    DR = mybir.MatmulPerfMode.DoubleRow

    nstrips = len(strip_blocks)
    groups = [
        (g * 4, min((g + 1) * 4, nstrips)) for g in range((nstrips + 3) // 4)
    ]

    nc = bacc.Bacc(None, name="ema_vbest")
    emb = nc.dram_tensor("emb", [NLOADS * P, 8 * D], f8, kind="ExternalInput")
    s = nc.dram_tensor("s", [P, S_COLS], f8, kind="ExternalInput")
    out = nc.dram_tensor(
        "out", [len(groups) * SLOT, 4 * D], bf16, kind="ExternalOutput"
    )

    with TileContext(nc) as tc:
        with (
            tc.tile_pool(name="sconst", bufs=1) as sconst,
            tc.tile_pool(name="epool", bufs=6) as epool,
            tc.tile_pool(name="opool", bufs=2) as opool,
            tc.tile_pool(name="ppool", bufs=4, space="PSUM") as ppool,
        ):
            # S block 2t..2t+2 of dim1 = [128, 2, 64] weights of m-block t
            s_tile = sconst.tile([P, 2 * NBLK, SLOT], f8)

            ets = {}

            def get_et(a):
                if a not in ets:
                    q = nc.sync if a % 2 == 0 else nc.scalar
                    q.dma_start(
                        out=s_tile[:, 8 * a : 8 * (a + 1), :],
                        in_=s[:, a * 8 * SLOT : (a + 1) * 8 * SLOT],
                    )
                    et = epool.tile([P, 8, D], f8, tag="et")
                    if a in (0, NLOADS - 1):
                        # half-loads: shorter pipeline fill/drain at the ends
                        q.dma_start(
                            out=et[:, 0:4, :],
                            in_=emb[a * P : (a + 1) * P, 0 : 4 * D],
                        )
                        q.dma_start(
                            out=et[:, 4:8, :],
                            in_=emb[a * P : (a + 1) * P, 4 * D : 8 * D],
                        )
                    else:
                        q.dma_start(out=et[:], in_=emb[a * P : (a + 1) * P, :])
                    ets[a] = et
                return ets[a]

            nstrips_ = len(strip_blocks)
            for g, (q0, q1) in enumerate(groups):
                ot = opool.tile([SLOT, 4 * D], bf16, tag="ot")
                for q in range(q0, q1):
                    pt = ppool.tile([SLOT, D], f32, tag="pt")
                    blocks = strip_blocks[q]
                    for r, t in enumerate(blocks):
                        et = get_et(t // 4)
                        lt = t % 4
                        for cl, ch in ((0, 512), (512, 768)):
                            nc.tensor.matmul(
                                pt[:, cl:ch],
                                s_tile[:, 2 * t : 2 * t + 2, :],
                                et[:, 2 * lt : 2 * lt + 2, cl:ch],
                                start=(r == 0), stop=(r == len(blocks) - 1),
                                perf_mode=DR,
                                tile_position=(0, 0),
                            )
                    # second-to-last strip's copy on scalar (all its loads are
                    # already issued, so no doorbell blocked behind the wait)
                    cp = nc.scalar if q == nstrips_ - 2 else nc.vector
                    if q == nstrips_ - 2:
                        cp.copy(ot[:, (q - q0) * D : (q - q0 + 1) * D], pt[:])
                    else:
                        cp.tensor_copy(
                            ot[:, (q - q0) * D : (q - q0 + 1) * D], pt[:]
                        )
                used = (q1 - q0) * D
                # last store on sync's HW-DGE queue (no loads left behind it)
                st = nc.sync if g == len(groups) - 1 else nc.gpsimd
                st.dma_start(
                    out=out[g * SLOT : (g + 1) * SLOT, 0:used],
                    in_=ot[:, 0:used],
                )
    nc.finalize()
    return nc


def _host_fallback(emb, lens):
    n = len(lens)
    ends = np.cumsum(lens)
    starts = ends - lens
    outp = np.zeros((n, 2 * D), dtype=np.float32)
    for i in range(n):
        L = int(lens[i])
        s0 = int(starts[i])
        if L >= 1:
            outp[i, D:] = emb[int(ends[i]) - 1]
            k = np.arange(L)
            w = np.where(
                k == L - 1,
                np.float32(0.0),
                np.where(
                    k == L - 2,
                    np.power(TAU, np.float32(L) - np.float32(2.0)),
                    (np.float32(1.0) - TAU) * np.power(TAU, k.astype(np.float32)),
                ),
            ).astype(np.float32)
            outp[i, :D] = w @ emb[s0 : s0 + L]
    return outp


def _prepare(lens):
    key = lens.tobytes()
    if key in _cache:
        return _cache[key]

    import ml_dtypes

    total = int(lens.sum())
    ends = np.cumsum(lens)
    starts = ends - lens
    plan = None
    if lens.min() >= 1:
        pos = np.arange(total)
        seg = np.searchsorted(ends, pos, side="right")
        k = pos - starts[seg]
        L = lens[seg]
        w_dev = np.where(
            k <= L - 3,
            (np.float32(1.0) - TAU) * np.power(TAU, k.astype(np.float32)),
            np.float32(0.0),
        ).astype(np.float32)
        ema_rows = np.nonzero(w_dev > 0)[0]
        if len(ema_rows) >= N_CORES * R_CORE:
            seg_ema = seg[ema_rows]
            # per-core, per-block fragment boundaries
            F = np.zeros((N_CORES, NBLK), dtype=np.int64)
            frag_info = [[None] * NBLK for _ in range(N_CORES)]
            for c in range(N_CORES):
                d = seg_ema[c * R_CORE : (c + 1) * R_CORE]
                for t in range(NBLK):
                    ds = d[t * 256 : (t + 1) * 256]
                    fs = np.concatenate(
                        ([0], np.flatnonzero(np.diff(ds)) + 1)
                    )
                    F[c, t] = len(fs)
                    frag_info[c][t] = (fs, ds)
            # common greedy strips: per-core sums all <= SLOT
            strip_blocks = []
            t0 = 0
            cur = np.zeros(N_CORES, dtype=np.int64)
            ok = True
            for t in range(NBLK):
                if F[:, t].max() > SLOT:
                    ok = False
                    break
                if (cur + F[:, t] > SLOT).any():
                    strip_blocks.append(list(range(t0, t)))
                    t0 = t
                    cur = F[:, t].copy()
                else:
                    cur += F[:, t]
            strip_blocks.append(list(range(t0, NBLK)))
            if ok:
                nstrips = len(strip_blocks)
                strip_of = np.zeros(NBLK, dtype=np.int64)
                for q, blks in enumerate(strip_blocks):
                    for t in blks:
                        strip_of[t] = q
                S = [np.zeros((P, S_COLS), dtype=np.float32)
                     for _ in range(N_CORES)]
                prev_rows, prev_dias = [], []
                rel = np.arange(256)
                for c in range(N_CORES):
                    for q, blks in enumerate(strip_blocks):
                        off = 0
                        for t in blks:
                            fs, ds = frag_info[c][t]
                            j = np.searchsorted(fs, rel, side="right") - 1
                            cols = (2 * t + rel // P) * SLOT + off + j
                            S[c][rel % P, cols] = np.float32(1.0)
                            for jj, a in enumerate(fs):
                                prev_rows.append(
                                    (c * nstrips + q) * SLOT + off + jj
                                )
                                prev_dias.append(int(ds[a]))
                            off += len(fs)
                S = [x.astype(ml_dtypes.float8_e4m3) for x in S]
                prev_rows = np.asarray(prev_rows, dtype=np.int64)
                prev_dias = np.asarray(prev_dias, dtype=np.int64)
                order = np.argsort(prev_dias, kind="stable")
                prev_rows, prev_dias = prev_rows[order], prev_dias[order]
                first_mask = np.ones(len(prev_dias), dtype=bool)
                first_mask[1:] = prev_dias[1:] != prev_dias[:-1]
                tail_rows = ema_rows[N_CORES * R_CORE :]
                nprog = _build_program(strip_blocks)
                plan = (
                    nprog, nstrips, ema_rows, w_dev, S,
                    (prev_rows, prev_dias, first_mask), tail_rows,
                )
    _cache[key] = plan
    return plan


def kernel(sentence_embeddings, lens):
    import ml_dtypes

    emb = np.ascontiguousarray(np.asarray(sentence_embeddings, dtype=np.float32))
    lens = np.asarray(lens, dtype=np.int32)

    plan = _prepare(lens)
    if plan is None:
        return _host_fallback(emb, lens)

    (nc, nstrips, ema_rows, w_dev, S,
     (prev_rows, prev_dias, first_mask), tail_rows) = plan
    from concourse.bass_utils import run_bass_kernel_spmd

    ge8 = (emb[ema_rows[: N_CORES * R_CORE]]
           * w_dev[ema_rows[: N_CORES * R_CORE]][:, None]
           ).astype(ml_dtypes.float8_e4m3)

    in_maps = []
    for c in range(N_CORES):
        x = ge8[c * R_CORE : (c + 1) * R_CORE]
        x = np.ascontiguousarray(
            x.reshape(NLOADS, 8, P, D).transpose(0, 2, 1, 3)
        ).reshape(NLOADS * P, 8 * D)
        in_maps.append({"emb": x, "s": S[c]})

    res = run_bass_kernel_spmd(nc, in_maps, core_ids=list(range(N_CORES)))
    kernel._last_results = res

    ngroups = (nstrips + 3) // 4
    o_cores = []
    for c in range(N_CORES):
        o = np.asarray(res.results[c]["out"]).astype(np.float32)
        o = o.reshape(ngroups, SLOT, 4, D).transpose(0, 2, 1, 3)
        o_cores.append(o.reshape(ngroups * 4, SLOT, D)[:nstrips])
    o_all = np.concatenate(o_cores, axis=0).reshape(-1, D)

    n = len(lens)
    ends = np.cumsum(lens)
    outp = np.zeros((n, 2 * D), dtype=np.float32)
    prev = outp[:, :D]
    prev[prev_dias[first_mask]] = o_all[prev_rows[first_mask]]
    nm = ~first_mask
    if nm.any():
        np.add.at(prev, prev_dias[nm], o_all[prev_rows[nm]])
    if len(tail_rows):
        segs = np.searchsorted(ends, tail_rows, side="right")
        np.add.at(prev, segs, emb[tail_rows] * w_dev[tail_rows][:, None])
    mask = lens >= 2
    Lf = lens[mask].astype(np.float32)
    prev[mask] += (TAU ** (Lf - np.float32(2.0)))[:, None] * emb[(ends - 2)[mask]]
    outp[:, D:] = emb[ends - 1]
    return outp
